# revision 3
# baseline (speedup 1.0000x reference)
"""MoE layer (top-2 routing, 8 experts) for Trainium2 across 8 NeuronCores.

Strategy: pair-sharded expert parallelism in bf16, software-pipelined.
  - Gate (x @ Wg, top-2 + softmax) on host (~0.03% of FLOPs). Experts are
    paired hot+cold by routed-token count; each pair is served by 2 cores
    with both experts' FFNs split along the hidden dim H (each core holds
    W1[:, h*2048:(h+1)*2048] / W2[h*2048:...] for BOTH experts -- 16 MB,
    same SBUF footprint as one full expert). Both cores of a pair process
    the union of the pair's routed tokens over their half-FFN; host sums
    the two partial outputs. Worst-core load drops from C_max = max expert
    count (1091) to (A_cap + B_cap)/2 ~ 1056 token-equivalents.
  - SOFTWARE PIPELINING (the big win, 308 -> 259 us): phase A (x@W1+gelu)
    of chunk c+1 interleaves with phase B (h@W2 * gate) of chunk c in an
    [A,A,B]x8 pattern, so the PE never sits at a bulk-synchronous phase
    boundary. Measured: each A|B boundary in the sequential schedule cost
    ~3-5 us of PE idle (psum-pool waits + HAM clock re-throttle).
  - Phase B has NO activation op (vector engine multiplies PSUM by the
    gate directly; b2 folded into the host combine -- exact algebra), so
    gelu is the only ACT table ever loaded.
  - Token tiles ride a 3-deep DMA ring prefetched >=1.5 blocks ahead.

fp8 (e4m3 DoubleRow) was evaluated and rejected: raw k-pair form is 2.37x
faster per FLOP but its 5.4e-2 max-rel error blows the 2e-2 gate, and
every first-order-exact fp8 scheme needs >=3 matmuls per 2 k-tiles --
slower than bf16 at measured issue rates (252.1 vs 212.8 ns per 512-col
MM). Measured HW exec time: ~259 us (bf16 column floor at the ~2.05 GHz
sustained P0 clock is ~250 us).
"""

import os
from contextlib import ExitStack

import ml_dtypes
import numpy as np

import concourse.bass as bass
import concourse.tile as tile
from concourse import bacc, mybir
from concourse.bass_utils import run_bass_kernel_spmd

try:  # pragma: no cover
    import antenv.axon_hooks  # noqa: F401
except ImportError:
    os.environ.setdefault("BASS_NEVER_TRACE", "1")

BF16 = ml_dtypes.bfloat16
D, H, O, E, TOPK = 1024, 4096, 1024, 8, 2
P = 128
N_CORES = 8
H2 = H // 2
N_D, N_H2, N_O = D // P, H2 // P, O // P  # 8, 16, 8

_CACHE: dict[tuple, bass.Bass] = {}


def _token_tiles(C):
    """Near-equal moving-dim chunks <= 512 (see baseline kernel notes)."""
    n_chunks = -(-C // 512)
    base, rem = divmod(C, n_chunks)
    tiles, t0 = [], 0
    for i in range(n_chunks):
        n = base + (1 if i < rem else 0)
        tiles.append((t0, n))
        t0 += n
    return tiles


def _build(A_cap: int, B_cap: int, iters: int = 1) -> bass.Bass:
    f32, bf16 = mybir.dt.float32, mybir.dt.bfloat16
    C2 = A_cap + B_cap
    nc = bacc.Bacc("TRN2", target_bir_lowering=False, debug=False,
                   num_devices=N_CORES)
    xt_d = nc.dram_tensor("xt", [D, C2], bf16, kind="ExternalInput").ap()
    w1a_d = nc.dram_tensor("w1a", [D, H2], bf16, kind="ExternalInput").ap()
    w1b_d = nc.dram_tensor("w1b", [D, H2], bf16, kind="ExternalInput").ap()
    w2a_d = nc.dram_tensor("w2a", [H2, O], bf16, kind="ExternalInput").ap()
    w2b_d = nc.dram_tensor("w2b", [H2, O], bf16, kind="ExternalInput").ap()
    b1a_d = nc.dram_tensor("b1a", [P, N_H2], f32, kind="ExternalInput").ap()
    b1b_d = nc.dram_tensor("b1b", [P, N_H2], f32, kind="ExternalInput").ap()
    g_d = nc.dram_tensor("g", [P, C2], f32, kind="ExternalInput").ap()
    yt_d = nc.dram_tensor("yt", [O, C2], f32, kind="ExternalOutput").ap()

    jobs = ([("a", t0, nt) for (t0, nt) in _token_tiles(A_cap)]
            + [("b", A_cap + t0, nt) for (t0, nt) in _token_tiles(B_cap)])

    with tile.TileContext(nc) as tc, ExitStack() as ctx:
        wpool = ctx.enter_context(tc.tile_pool(name="weights", bufs=1))
        xpool = ctx.enter_context(tc.tile_pool(name="xin", bufs=1))
        hpool = ctx.enter_context(tc.tile_pool(name="hts", bufs=34))
        ppool1 = ctx.enter_context(tc.tile_pool(name="ps1", bufs=2, space="PSUM"))
        ppool2 = ctx.enter_context(tc.tile_pool(name="ps2", bufs=2, space="PSUM"))
        ypool = ctx.enter_context(tc.tile_pool(name="yout", bufs=2))

        w1_sb = {"a": wpool.tile([P, N_D, H2], bf16, name="w1a"),
                 "b": wpool.tile([P, N_D, H2], bf16, name="w1b")}
        w2_sb = {"a": wpool.tile([P, N_H2, O], bf16, name="w2a"),
                 "b": wpool.tile([P, N_H2, O], bf16, name="w2b")}
        b1_sb = {"a": wpool.tile([P, N_H2], f32, name="b1a"),
                 "b": wpool.tile([P, N_H2], f32, name="b1b")}
        g_sb = wpool.tile([P, C2], f32)
        w1_dram = {"a": w1a_d, "b": w1b_d}
        w2_dram = {"a": w2a_d, "b": w2b_d}

        # xt tiles ride a 3-deep ring, prefetched >=1.5 blocks ahead of
        # use so no phase block ever waits on the DMA queue.
        RING = 3
        nj = len(jobs)
        xt_tiles = [xpool.tile([P, N_D, 512], bf16, name=f"xt{r}")
                    for r in range(RING)]

        def fetch(ji):
            (ex_, t0_, nt_) = jobs[ji]
            xt_t = xt_tiles[ji % RING]
            for d in range(N_D):
                nc.sync.dma_start(out=xt_t[:, d, :nt_],
                                  in_=xt_d[d * P:(d + 1) * P, t0_:t0_ + nt_])

        for ji in range(min(RING, nj)):
            fetch(ji)
        for hc in range(H2 // 512):
            c0, c1 = hc * 512, (hc + 1) * 512
            for d in range(N_D):
                nc.sync.dma_start(out=w1_sb["a"][:, d, c0:c1],
                                  in_=w1a_d[d * P:(d + 1) * P, c0:c1])
            if hc == 0:
                nc.sync.dma_start(out=b1_sb["a"][:], in_=b1a_d[:])
        for h in range(N_H2):
            nc.sync.dma_start(out=w2_sb["a"][:, h, :],
                              in_=w2a_d[h * P:(h + 1) * P, :])
        nc.sync.dma_start(out=g_sb[:], in_=g_d[:])
        for hc in range(H2 // 512):
            c0, c1 = hc * 512, (hc + 1) * 512
            for d in range(N_D):
                nc.sync.dma_start(out=w1_sb["b"][:, d, c0:c1],
                                  in_=w1b_d[d * P:(d + 1) * P, c0:c1])
            if hc == 0:
                nc.sync.dma_start(out=b1_sb["b"][:], in_=b1b_d[:])
        for h in range(N_H2):
            nc.sync.dma_start(out=w2_sb["b"][:, h, :],
                              in_=w2b_d[h * P:(h + 1) * P, :])

        gelu = mybir.ActivationFunctionType.Gelu
        copy = mybir.ActivationFunctionType.Identity

        loop_ctx = ExitStack()
        if iters > 1:
            loop_ctx.enter_context(tc.For_i(0, iters, 1))
        ctx.enter_context(loop_ctx)

        # Software-pipelined schedule: phase A of chunk c+1 interleaves
        # with phase B of chunk c ([A,A,B] x 8 per block), so the PE never
        # hits a bulk-synchronous A|B phase boundary mid-kernel. h tiles of
        # two chunks are live at once (hpool bufs >= 34).
        def a_group(ji, m):
            (ex, t0, nt) = jobs[ji]
            ps = ppool1.tile([P, 512], f32, tag="ps1")
            for d in range(N_D):
                nc.tensor.matmul(ps[:, :nt],
                                 lhsT=w1_sb[ex][:, d, m * P:(m + 1) * P],
                                 rhs=xt_tiles[ji % RING][:, d, :nt],
                                 start=(d == 0), stop=(d == N_D - 1))
            ht = hpool.tile([P, 512], bf16, tag="ht")
            nc.scalar.activation(ht[:, :nt], ps[:, :nt], gelu,
                                 bias=b1_sb[ex][:, m:m + 1])
            return ht

        def b_group(ji, o, hts):
            (ex, t0, nt) = jobs[ji]
            ps2 = ppool2.tile([P, 512], f32, tag="ps2")
            for h in range(N_H2):
                nc.tensor.matmul(ps2[:, :nt],
                                 lhsT=w2_sb[ex][:, h, o * P:(o + 1) * P],
                                 rhs=hts[h][:, :nt],
                                 start=(h == 0), stop=(h == N_H2 - 1))
            ym = ypool.tile([P, 512], f32, tag="ym")
            nc.vector.tensor_mul(ym[:, :nt], ps2[:, :nt],
                                 g_sb[:, t0:t0 + nt])
            nc.sync.dma_start(out=yt_d[o * P:(o + 1) * P, t0:t0 + nt],
                              in_=ym[:, :nt])

        # prologue: phase A of job 0
        hts_cur = [a_group(0, m) for m in range(N_H2)]
        for c in range(nj):
            # ring prefetch: forward fetch lands right after the tile's
            # previous reader; wrap fetches (next For_i iteration) are
            # emitted at the block AFTER the tile's last same-iteration
            # reader (job w's A-phase runs in block w-1).
            nxt = c + RING
            if nxt < nj:
                fetch(nxt)
            if iters > 1:
                for w_ in range(RING):
                    if w_ + RING * ((nj - 1 - w_) // RING) == c:
                        fetch(w_)
            if c + 1 < nj:
                hts_next = []
                for o in range(N_O):
                    hts_next.append(a_group(c + 1, 2 * o))
                    hts_next.append(a_group(c + 1, 2 * o + 1))
                    b_group(c, o, hts_cur)
                hts_cur = hts_next
            else:
                for o in range(N_O):
                    b_group(c, o, hts_cur)
    nc.compile()
    return nc


def _prepare(x, Wg, W1, b1, W2, b2):
    """Host gating + pair assignment + per-core input maps."""
    B, S, Dx = x.shape
    assert Dx == D and Wg.shape == (D, E)
    T = B * S
    xf = np.ascontiguousarray(x.reshape(T, D), dtype=np.float32)
    logits = xf.astype(np.float64) @ Wg.astype(np.float64)
    top_i = np.argpartition(-logits, TOPK - 1, axis=1)[:, :TOPK]
    lv = np.take_along_axis(logits, top_i, axis=1)
    lv -= lv.max(axis=1, keepdims=True)
    ex_ = np.exp(lv)
    w = ex_ / ex_.sum(axis=1, keepdims=True)

    flat_e = top_i.reshape(-1)
    flat_w = w.reshape(-1)
    counts = np.bincount(flat_e, minlength=E)

    order = np.argsort(-counts, kind="stable")
    pairs = [(int(order[i]), int(order[E - 1 - i])) for i in range(E // 2)]
    A_cap = max(1024, int(max(counts[a] for a, _ in pairs)))
    B_cap = max(512, int(max(counts[b] for _, b in pairs)))
    C2 = A_cap + B_cap

    xt_bf = np.ascontiguousarray(xf.T).astype(BF16)  # [D, T]
    W1b_ = W1.astype(BF16)
    W2b_ = W2.astype(BF16)

    in_maps = []
    glob = np.empty(2 * T, dtype=np.int64)  # pair-slot -> row in stacked Y
    for p, (ea, eb) in enumerate(pairs):
        xt_p = np.zeros((D, C2), dtype=BF16)
        g_p = np.zeros((C2,), dtype=np.float32)
        for slot_base, e in ((0, ea), (A_cap, eb)):
            sel = np.nonzero(flat_e == e)[0]
            tok = sel >> 1
            n = len(sel)
            xt_p[:, slot_base:slot_base + n] = xt_bf[:, tok]
            g_p[slot_base:slot_base + n] = flat_w[sel]
            glob[sel] = p * C2 + slot_base + np.arange(n)
        g_bc = np.ascontiguousarray(np.broadcast_to(g_p, (P, C2)))
        for hf in range(2):
            sl1 = slice(hf * H2, (hf + 1) * H2)
            in_maps.append({
                "xt": xt_p,
                "w1a": np.ascontiguousarray(W1b_[ea][:, sl1]),
                "w1b": np.ascontiguousarray(W1b_[eb][:, sl1]),
                "w2a": np.ascontiguousarray(W2b_[ea][sl1, :]),
                "w2b": np.ascontiguousarray(W2b_[eb][sl1, :]),
                "b1a": np.ascontiguousarray(
                    np.asarray(b1[ea][sl1], np.float32).reshape(N_H2, P).T),
                "b1b": np.ascontiguousarray(
                    np.asarray(b1[eb][sl1], np.float32).reshape(N_H2, P).T),
                "g": g_bc,
            })
    b2f = np.asarray(b2, np.float32)
    corr = (w[:, 0:1] * b2f[top_i[:, 0]]
            + w[:, 1:2] * b2f[top_i[:, 1]]).astype(np.float32)  # [T, O]
    return in_maps, (glob, corr), (A_cap, B_cap), B, S


def _get_nc(caps, iters: int = 1) -> bass.Bass:
    key = (caps, iters)
    nc = _CACHE.get(key)
    if nc is None:
        nc = _CACHE[key] = _build(caps[0], caps[1], iters)
    return nc


def _combine(results, glob_corr, caps, B, S):
    glob, corr = glob_corr
    C2 = caps[0] + caps[1]
    # pair p partial sum: cores 2p and 2p+1
    Y = np.stack([np.asarray(results[2 * p]["yt"]).T
                  + np.asarray(results[2 * p + 1]["yt"]).T
                  for p in range(E // 2)])  # [4, C2, O]
    Yflat = Y.reshape(4 * C2, O)
    out = Yflat[glob[0::2]] + Yflat[glob[1::2]] + corr
    return out.reshape(B, S, O).astype(np.float32, copy=False)


def kernel(x, Wg, W1, b1, W2, b2):
    in_maps, glob, caps, B, S = _prepare(x, Wg, W1, b1, W2, b2)
    nc = _get_nc(caps)
    res = run_bass_kernel_spmd(nc, in_maps, core_ids=list(range(N_CORES)))
    return _combine(res.results, glob, caps, B, S)
